# revision 1
# baseline (speedup 1.0000x reference)
"""Multi-head self-attention with RoPE on 8 Trainium2 NeuronCores.

Sharding: core c = batch(c // 4) x head-group(c % 4) -> 4 heads per core.
Each core computes attention for its 4 heads and a partial O-projection
(full [S, D] output restricted to its 256 input features); the host sums
the 4 partials per batch (in fp32, from bf16 device partials).

Device layout tricks:
  * x is transposed on host -> xT [D, S] bf16; all projection matmuls
    contract over partitions without any on-device transpose.
  * Wq/Wk rows are permuted on host so even rope dims (E) and odd rope
    dims (O) of the 4 heads land in two separate 128-row projection
    outputs. RoPE then becomes lane-aligned elementwise DVE ops, and the
    Q.K contraction (invariant to the shared permutation) is done as two
    accumulating K=32 matmuls per head at distinct PE row-groups.
  * Scores are computed transposed (S_T[k, q]) so P_T feeds the PV matmul
    as the moving operand; a ones-column appended to V accumulates the
    softmax denominator in the same matmul. Softmax skips the max
    subtraction (scores are bounded ~|5|), exactly like exp-sum-divide.
  * All matmul operands are bf16 (measured ~1.4-1.8x faster than f32r on
    HW); psum accumulation stays fp32. End-to-end rel err ~3e-3.
  * exp is issued one PSUM bank at a time ([128,512]); a single AP
    spanning two banks measured ~5x slower on HW. Score tiles are
    single-chunk single-bank with 4-deep buffering so scores(c+1)
    overlap exp(c).
  * softmax denominator reciprocal uses the 1-instruction approx
    (~51 ULP) instead of the ~6 cycle/elem iterative divide; V blocks
    are padded to 128 rows (ones col first) so the denominator lands at
    acc partition 0 where the custom-DVE op works.
  * projection(sb), O-projection(sb-1) and attention(sb) are emitted in
    one fused pipelined loop (PSUM: proj 2 banks + scores 4 + accs 2) so
    projection PE work overlaps the attention ACT/normalize tail.
"""

import os
import sys

sys.path.insert(0, "/opt/trn_rl_repo")

from contextlib import ExitStack

import numpy as np

import concourse.bass as bass
import concourse.tile as tile
from concourse import bacc, mybir
from concourse.bass_utils import run_bass_kernel_spmd

B = 2
S = 4096
D = 1024
NH = 16
DK = 64
HPC = 4  # heads per core
N_CORES = 8
THETA = 10000.0
SBLK = 512  # s-block / q-block width
NSB = S // SBLK
KC = 128  # k chunk
F32 = mybir.dt.float32
BF16 = mybir.dt.bfloat16
EXP = mybir.ActivationFunctionType.Exp

_PROGRAM = None


def _emit(nc, loop_n=1):
    xT = nc.dram_tensor("xT", [D, S], BF16, kind="ExternalInput").ap()
    wqeT = nc.dram_tensor("wqeT", [D, 128], BF16, kind="ExternalInput").ap()
    wqoT = nc.dram_tensor("wqoT", [D, 128], BF16, kind="ExternalInput").ap()
    wkeT = nc.dram_tensor("wkeT", [D, 128], BF16, kind="ExternalInput").ap()
    wkoT = nc.dram_tensor("wkoT", [D, 128], BF16, kind="ExternalInput").ap()
    wvT = nc.dram_tensor("wvT", [D, 256], BF16, kind="ExternalInput").ap()
    woT = nc.dram_tensor("woT", [256, D], BF16, kind="ExternalInput").ap()
    cos4 = nc.dram_tensor("cos4", [128, S], F32, kind="ExternalInput").ap()
    sin4 = nc.dram_tensor("sin4", [128, S], F32, kind="ExternalInput").ap()
    masks = nc.dram_tensor("masks", [128, 128], BF16, kind="ExternalInput").ap()
    outp = nc.dram_tensor("out", [S, D], BF16, kind="ExternalOutput").ap()
    dump = os.environ.get("K_DUMP") == "1"
    if dump:
        d_qtE = nc.dram_tensor("d_qtE", [128, S], BF16, kind="ExternalOutput").ap()
        d_ktE = nc.dram_tensor("d_ktE", [128, S], BF16, kind="ExternalOutput").ap()
        d_vbig = nc.dram_tensor("d_vbig", [128, 32 * 512], BF16, kind="ExternalOutput").ap()
        d_pt = nc.dram_tensor("d_pt", [128, SBLK], BF16, kind="ExternalOutput").ap()
        d_at = nc.dram_tensor("d_at", [128, SBLK], BF16, kind="ExternalOutput").ap()
        d_rec = nc.dram_tensor("d_rec", [1, SBLK], F32, kind="ExternalOutput").ap()

    with tile.TileContext(nc) as tc, ExitStack() as ctx:
        wpool = ctx.enter_context(tc.tile_pool(name="w", bufs=1))
        xpool = ctx.enter_context(tc.tile_pool(name="x", bufs=24))
        qkpool = ctx.enter_context(tc.tile_pool(name="qk", bufs=1))
        vpool = ctx.enter_context(tc.tile_pool(name="v", bufs=1))
        trig = ctx.enter_context(tc.tile_pool(name="trig", bufs=3))
        tmp = ctx.enter_context(tc.tile_pool(name="tmp", bufs=4))
        ppool = ctx.enter_context(tc.tile_pool(name="p", bufs=16))
        apool = ctx.enter_context(tc.tile_pool(name="a", bufs=8))
        rpool = ctx.enter_context(tc.tile_pool(name="r", bufs=4))
        rbpool = ctx.enter_context(tc.tile_pool(name="rb", bufs=4))

        # ---- persistent SBUF tensors ----
        w_qe = wpool.tile([128, 8 * 128], BF16, tag="wqe")
        w_qo = wpool.tile([128, 8 * 128], BF16, tag="wqo")
        w_ke = wpool.tile([128, 8 * 128], BF16, tag="wke")
        w_ko = wpool.tile([128, 8 * 128], BF16, tag="wko")
        w_v = wpool.tile([128, 8 * 256], BF16, tag="wv")
        w_o0 = wpool.tile([128, D], BF16, tag="wo0")
        w_o1 = wpool.tile([128, D], BF16, tag="wo1")
        mask_t = wpool.tile([128, 128], BF16, tag="mask")

        for dst, src, nf in (
            (w_qe, wqeT, 128),
            (w_qo, wqoT, 128),
            (w_ke, wkeT, 128),
            (w_ko, wkoT, 128),
            (w_v, wvT, 256),
        ):
            nc.sync.dma_start(
                dst[:].rearrange("p (i f) -> p i f", i=8),
                src.rearrange("(i p) f -> p i f", p=128),
            )
        nc.sync.dma_start(w_o0[:], woT[0:128, :])
        nc.sync.dma_start(w_o1[:], woT[128:256, :])
        nc.sync.dma_start(mask_t[:], masks)

        qtE = qkpool.tile([128, S], BF16, tag="qtE")
        qtO = qkpool.tile([128, S], BF16, tag="qtO")
        ktE = qkpool.tile([128, S], BF16, tag="ktE")
        ktO = qkpool.tile([128, S], BF16, tag="ktO")
        # V per head padded to 96 rows: [1 | 0*31 | V_h (64)] x4 per chunk.
        # The leading ones-column accumulates the softmax denominator at acc
        # partition 0 (custom-DVE recip needs base partition 0); the zero pad
        # keeps the V rows 32-aligned for the PSUM normalize read.
        vbig = vpool.tile([128, 32 * 512], BF16, tag="vbig")
        nc.vector.memset(vbig[:], 0.0)
        nc.vector.memset(
            vbig[:].rearrange("p (c h f) -> p (c h) f", c=32, h=4)[:, :, 0:1], 1.0
        )  # f = 128 rows per head block

        # ---- fused loop: proj(sb) ; oproj(sb-1) ; attention(sb) ----
        # proj(sb+1) PE work overlaps attention(sb)'s ACT/normalize tail;
        # oproj(k) is emitted one iteration late so its matmuls never
        # head-of-line-block the PE queue on normalize(k).
        loop_ctx = tc.For_i(0, loop_n, 1) if loop_n > 1 else None
        if loop_ctx is not None:
            loop_ctx.__enter__()
        dump = dump and loop_n == 1
        with (
            tc.tile_pool(name="projps", bufs=2, space="PSUM") as projps,
            tc.tile_pool(name="sps", bufs=4, space="PSUM") as sps,
            tc.tile_pool(name="accps", bufs=2, space="PSUM") as accps,
        ):

            def emit_proj(sb):
                scol = slice(sb * SBLK, (sb + 1) * SBLK)
                xt = []
                for i in range(8):
                    t = xpool.tile([128, SBLK], BF16, tag="xt")
                    nc.sync.dma_start(t[:], xT[i * 128 : (i + 1) * 128, scol])
                    xt.append(t)
                cos_t = trig.tile([128, SBLK], F32, tag="cos")
                sin_t = trig.tile([128, SBLK], F32, tag="sin")
                nc.sync.dma_start(cos_t[:], cos4[:, scol])
                nc.sync.dma_start(sin_t[:], sin4[:, scol])

                for pair, (ne, we, no, wo) in enumerate(
                    (("qe", w_qe, "qo", w_qo), ("ke", w_ke, "ko", w_ko))
                ):
                    pe = projps.tile([128, SBLK], F32, tag="proj", name=f"pe{sb}_{pair}")
                    po = projps.tile([128, SBLK], F32, tag="proj", name=f"po{sb}_{pair}")
                    for dst, w in ((pe, we), (po, wo)):
                        for i in range(8):
                            nc.tensor.matmul(
                                dst[:],
                                w[:, i * 128 : (i + 1) * 128],
                                xt[i][:],
                                start=(i == 0),
                                stop=(i == 7),
                            )
                    dE, dO = (qtE, qtO) if pair == 0 else (ktE, ktO)
                    t1 = tmp.tile([128, SBLK], F32, tag="t1")
                    t2 = tmp.tile([128, SBLK], F32, tag="t2")
                    nc.vector.tensor_mul(t1[:], pe[:], cos_t[:])
                    nc.vector.tensor_mul(t2[:], po[:], sin_t[:])
                    nc.vector.tensor_sub(dE[:, scol], t1[:], t2[:])
                    t3 = tmp.tile([128, SBLK], F32, tag="t1")
                    t4 = tmp.tile([128, SBLK], F32, tag="t2")
                    nc.vector.tensor_mul(t3[:], pe[:], sin_t[:])
                    nc.vector.tensor_mul(t4[:], po[:], cos_t[:])
                    nc.vector.tensor_add(dO[:, scol], t3[:], t4[:])

                # V for the 4 k-chunks of this s-block
                for ss in range(4):
                    vp = projps.tile([128, 256], F32, tag="proj", name=f"vp{sb}_{ss}")
                    for i in range(8):
                        nc.tensor.matmul(
                            vp[:],
                            xt[i][:, ss * 128 : (ss + 1) * 128],
                            w_v[:, i * 256 : (i + 1) * 256],
                            start=(i == 0),
                            stop=(i == 7),
                        )
                    kc = sb * 4 + ss
                    nc.vector.tensor_copy(
                        vbig[:, kc * 512 : (kc + 1) * 512].rearrange(
                            "p (h f) -> p h f", h=4
                        )[:, :, 64:128],
                        vp[:].rearrange("p (h f) -> p h f", h=4),
                    )

            def emit_attn(qb, a_tiles):
                nk = 4 * qb + 4
                for hp in range(2):
                    heads = (2 * hp, 2 * hp + 1)
                    accs = [
                        accps.tile([128, SBLK], F32, tag="acc", name=f"acc{qb}_{h}")
                        for h in heads
                    ]
                    for c in range(nk):
                        d = max(0, (c - 4 * qb)) * KC
                        sp_pair = [
                            sps.tile([128, SBLK], F32, tag="s", name=f"s{qb}_{c}_{h}")
                            for h in heads
                        ]
                        for kt, qt, st0, st1 in (
                            (ktE, qtE, True, False),
                            (ktO, qtO, False, True),
                        ):
                            for hi, h in enumerate(heads):
                                hr = slice(h * 32, (h + 1) * 32)
                                nc.tensor.matmul(
                                    sp_pair[hi][:, d:SBLK],
                                    kt[hr, c * KC : (c + 1) * KC],
                                    qt[hr, qb * SBLK + d : (qb + 1) * SBLK],
                                    start=st0,
                                    stop=st1,
                                    tile_position=(h * 32, 0),
                                )
                        pts = []
                        for hi, h in enumerate(heads):
                            pt = ppool.tile([128, SBLK], BF16, tag="pt")
                            nc.scalar.activation(
                                pt[:, d:SBLK], sp_pair[hi][:, d:SBLK], EXP, scale=0.125
                            )
                            if c >= 4 * qb:
                                nc.vector.tensor_mul(
                                    pt[:, d : d + KC], pt[:, d : d + KC], mask_t[:]
                                )
                            pts.append(pt)
                        if dump and qb == 1 and hp == 0 and c == 0:
                            nc.sync.dma_start(d_pt, pts[0][:])
                        for hi, h in enumerate(heads):
                            nc.tensor.matmul(
                                accs[hi][0:128, d:SBLK],
                                vbig[:, c * 512 + h * 128 : c * 512 + (h + 1) * 128],
                                pts[hi][:, d:SBLK],
                                start=(c == 0),
                                stop=(c == nk - 1),
                            )
                    for hi, h in enumerate(heads):
                        recip = rpool.tile([1, SBLK], F32, tag="recip")
                        nc.vector.reciprocal_approx_fast(recip[:], accs[hi][0:1, :])
                        if dump and qb == 1 and hi == 0 and hp == 0:
                            nc.sync.dma_start(d_rec, recip[:])
                        rb = rbpool.tile([64, SBLK], F32, tag="rb")
                        nc.gpsimd.partition_broadcast(rb[:], recip[0:1, :])
                        nc.vector.tensor_mul(
                            a_tiles[h // 2][(h % 2) * 64 : (h % 2 + 1) * 64, :],
                            accs[hi][64:128, :],
                            rb[:],
                        )
                if dump and qb == 1:
                    nc.sync.dma_start(d_at, a_tiles[0][:])

            def emit_oproj(qb, a_tiles):
                for ss in range(4):
                    r0 = qb * SBLK + ss * 128
                    for ob in range(2):
                        op = projps.tile(
                            [128, 512], F32, tag="proj", name=f"op{qb}_{ss}_{ob}"
                        )
                        nc.tensor.matmul(
                            op[:],
                            a_tiles[0][:, ss * 128 : (ss + 1) * 128],
                            w_o0[:, ob * 512 : (ob + 1) * 512],
                            start=True,
                            stop=False,
                        )
                        nc.tensor.matmul(
                            op[:],
                            a_tiles[1][:, ss * 128 : (ss + 1) * 128],
                            w_o1[:, ob * 512 : (ob + 1) * 512],
                            start=False,
                            stop=True,
                        )
                        osb = ppool.tile(
                            [128, 512], BF16, tag="osb", bufs=2, name=f"osb{qb}_{ss}_{ob}"
                        )
                        nc.vector.tensor_copy(osb[:], op[:])
                        nc.sync.dma_start(
                            outp[r0 : r0 + 128, ob * 512 : (ob + 1) * 512], osb[:]
                        )

            at_all = [
                [
                    apool.tile([128, SBLK], BF16, tag="a", name=f"a{qb}_{i}", bufs=4)
                    for i in range(2)
                ]
                for qb in range(NSB)
            ]
            # proj runs one full iteration ahead of attention, so
            # attention's first scores never wait on fresh rope output;
            # oproj first: its psum slots were freed an iteration ago
            emit_proj(0)
            emit_proj(1)
            emit_attn(0, at_all[0])
            for sb in range(1, NSB):
                emit_oproj(sb - 1, at_all[sb - 1])
                if sb + 1 < NSB:
                    emit_proj(sb + 1)
                emit_attn(sb, at_all[sb])
            emit_oproj(NSB - 1, at_all[NSB - 1])
        if loop_ctx is not None:
            loop_ctx.__exit__(None, None, None)


def _build(loop_n=1):
    global _PROGRAM
    if loop_n != 1:
        nc = bacc.Bacc(
            "TRN2", target_bir_lowering=False, debug=False, num_devices=N_CORES
        )
        _emit(nc, loop_n)
        nc.compile()
        return nc
    if _PROGRAM is None:
        nc = bacc.Bacc(
            "TRN2", target_bir_lowering=False, debug=False, num_devices=N_CORES
        )
        _emit(nc)
        nc.compile()
        _PROGRAM = nc
    return _PROGRAM


def _rope_caches():
    j = np.arange(0, DK, 2, dtype=np.float32) / np.float32(DK)
    freqs = (1.0 / THETA**j).astype(np.float32)  # [32]
    t = np.arange(S, dtype=np.float32)
    ang = np.outer(t, freqs).astype(np.float32)  # [S, 32]
    return np.cos(ang), np.sin(ang)


def _make_masks():
    kk = np.arange(128)[:, None]
    jj = np.arange(128)[None, :]
    return (kk <= jj).astype(np.float32)


def _to_bf16(a):
    import ml_dtypes

    return np.asarray(a, dtype=np.float32).astype(ml_dtypes.bfloat16)


def _make_in_maps(x, token_positions, Wq, Wk, Wv, Wo):
    x = np.asarray(x, dtype=np.float32)
    token_positions = np.asarray(token_positions)
    Wq, Wk, Wv, Wo = (np.asarray(w, dtype=np.float32) for w in (Wq, Wk, Wv, Wo))

    cos_c, sin_c = _rope_caches()
    masks = _to_bf16(_make_masks())

    in_maps = []
    for c in range(N_CORES):
        b, g = divmod(c, 4)
        heads = [g * HPC + hh for hh in range(HPC)]
        rows_e = np.concatenate([h * DK + np.arange(0, DK, 2) for h in heads])
        rows_o = rows_e + 1
        rows_v = np.concatenate([h * DK + np.arange(DK) for h in heads])

        pos = np.asarray(token_positions[b], dtype=np.int64)
        cosb = np.ascontiguousarray(cos_c[pos].T)  # [32, S]
        sinb = np.ascontiguousarray(sin_c[pos].T)

        in_maps.append(
            {
                "xT": _to_bf16(np.ascontiguousarray(x[b].T)),
                "wqeT": _to_bf16(np.ascontiguousarray(Wq[rows_e].T)),
                "wqoT": _to_bf16(np.ascontiguousarray(Wq[rows_o].T)),
                "wkeT": _to_bf16(np.ascontiguousarray(Wk[rows_e].T)),
                "wkoT": _to_bf16(np.ascontiguousarray(Wk[rows_o].T)),
                "wvT": _to_bf16(np.ascontiguousarray(Wv[rows_v].T)),
                "woT": _to_bf16(np.ascontiguousarray(Wo[:, rows_v].T)),
                "cos4": np.ascontiguousarray(np.tile(cosb, (4, 1))),
                "sin4": np.ascontiguousarray(np.tile(sinb, (4, 1))),
                "masks": masks,
            }
        )
    return in_maps


def kernel(x, token_positions, Wq, Wk, Wv, Wo):
    nc = _build()
    in_maps = _make_in_maps(x, token_positions, Wq, Wk, Wv, Wo)
    res = run_bass_kernel_spmd(nc, in_maps, list(range(N_CORES)))
    out = np.zeros((B, S, D), dtype=np.float32)
    for c in range(N_CORES):
        out[c // 4] += np.asarray(res.results[c]["out"], dtype=np.float32)
    return out



# revision 9
# speedup vs baseline: 1.1578x; 1.1578x over previous
"""Multi-head self-attention with RoPE on 8 Trainium2 NeuronCores.

Sharding: core c = batch(c // 4) x head-group(c % 4) -> 4 heads per core.
Each core computes attention for its 4 heads and a partial O-projection
(full [S, D] output restricted to its 256 input features); the host sums
the 4 partials per batch (in fp32, from bf16 device partials).

v2 layout (vs v1):
  * Q/K live in two [128, S] tiles per tensor: tile01 = heads {0,1},
    tile23 = {2,3}, rows per head = [E(32) | O(32)] (rope-even dims then
    odd dims). A head's score matmul is then a single 64-contraction
    matmul; the two heads of a tile run concurrently at tile_position
    (0,0)/(64,0) (measured 257ns per head-pair vs 930ns for the v1
    E/O-accumulate pattern).
  * RoPE needs E/O cross-partition arithmetic, which DVE can't do
    lane-misaligned; instead the projection is computed TWICE per tile:
    P1 with the interleaved weight, P2 with its 32-row-block-swapped
    copy (so P2 = swap32(P1), by PE). Then out = P1*cc + P2*ss with
    cc = [cos]*4 rows, ss = [-sin,+sin,-sin,+sin] rows: 3 full-width
    DVE ops, all lane-aligned.
  * exp is issued per-head over PAIRS of full k-chunks as one [128,1024]
    activate spanning 2 PSUM banks (measured 1086ns vs 2x625ns);
    partial (diagonal) chunks keep single [128, 512-d] activates.
  * Scores are computed transposed (S_T[k, q]) so P_T feeds the PV matmul
    as the moving operand; a ones-column appended to V accumulates the
    softmax denominator in the same matmul. Softmax skips the max
    subtraction (scores are bounded ~|5|).
  * All matmul operands are bf16; psum accumulation stays fp32.
  * softmax denominator reciprocal uses the 1-instruction approx
    custom-DVE op; V blocks are padded to 128 rows (ones col first) so
    the denominator lands at acc partition 0.
  * projection(sb), O-projection(sb-1) and attention(sb) are emitted in
    one fused pipelined loop (PSUM: proj 2 banks + scores 2x2 + accs 2).
"""

import os
import sys

sys.path.insert(0, "/opt/trn_rl_repo")

from contextlib import ExitStack

import numpy as np

import concourse.bass as bass
import concourse.tile as tile
from concourse import bacc, mybir
from concourse.bass_utils import run_bass_kernel_spmd

B = 2
S = 4096
D = 1024
NH = 16
DK = 64
HPC = 4  # heads per core
N_CORES = 8
THETA = 10000.0
SBLK = 512  # s-block / q-block width
NSB = S // SBLK
KC = 128  # k chunk
F32 = mybir.dt.float32
BF16 = mybir.dt.bfloat16
EXP = mybir.ActivationFunctionType.Exp

_PROGRAM = None


def _emit(nc, loop_n=1):
    xT = nc.dram_tensor("xT", [D, S], BF16, kind="ExternalInput").ap()
    # interleaved qk projection weights: per head-pair tile, rows =
    # [h_even E32 | h_even O32 | h_odd E32 | h_odd O32]; the *s variants
    # are the 32-row-block-swapped copies.
    wq = {}
    for t in ("q01", "q01s", "q23", "q23s", "k01", "k01s", "k23", "k23s"):
        wq[t] = nc.dram_tensor(f"w{t}T", [D, 128], BF16, kind="ExternalInput").ap()
    wvT = nc.dram_tensor("wvT", [D, 256], BF16, kind="ExternalInput").ap()
    woT = nc.dram_tensor("woT", [256, D], BF16, kind="ExternalInput").ap()
    cc4 = nc.dram_tensor("cc4", [128, S], F32, kind="ExternalInput").ap()
    ss4 = nc.dram_tensor("ss4", [128, S], F32, kind="ExternalInput").ap()
    masks = nc.dram_tensor("masks", [128, 128], BF16, kind="ExternalInput").ap()
    outp = nc.dram_tensor("out", [S, D], BF16, kind="ExternalOutput").ap()

    with tile.TileContext(nc) as tc, ExitStack() as ctx:
        wpool = ctx.enter_context(tc.tile_pool(name="w", bufs=1))
        xpool = ctx.enter_context(tc.tile_pool(name="x", bufs=24))
        qkpool = ctx.enter_context(tc.tile_pool(name="qk", bufs=1))
        vpool = ctx.enter_context(tc.tile_pool(name="v", bufs=1))
        trig = ctx.enter_context(tc.tile_pool(name="trig", bufs=3))
        tmp = ctx.enter_context(tc.tile_pool(name="tmp", bufs=4))
        ppool = ctx.enter_context(tc.tile_pool(name="p", bufs=16))
        apool = ctx.enter_context(tc.tile_pool(name="a", bufs=8))
        rpool = ctx.enter_context(tc.tile_pool(name="r", bufs=4))
        rbpool = ctx.enter_context(tc.tile_pool(name="rb", bufs=4))

        # ---- persistent SBUF tensors ----
        wt = {}
        for t in ("q01", "q01s", "q23", "q23s", "k01", "k01s", "k23", "k23s"):
            wt[t] = wpool.tile([128, 8 * 128], BF16, tag=f"w{t}", name=f"wt_{t}")
            nc.sync.dma_start(
                wt[t][:].rearrange("p (i f) -> p i f", i=8),
                wq[t].rearrange("(i p) f -> p i f", p=128),
            )
        w_v = wpool.tile([128, 8 * 256], BF16, tag="wv")
        nc.sync.dma_start(
            w_v[:].rearrange("p (i f) -> p i f", i=8),
            wvT.rearrange("(i p) f -> p i f", p=128),
        )
        w_o0 = wpool.tile([128, D], BF16, tag="wo0")
        w_o1 = wpool.tile([128, D], BF16, tag="wo1")
        mask_t = wpool.tile([128, 128], BF16, tag="mask")
        nc.sync.dma_start(w_o0[:], woT[0:128, :])
        nc.sync.dma_start(w_o1[:], woT[128:256, :])
        nc.sync.dma_start(mask_t[:], masks)

        # q/k tiles in interleaved layout: [h_lo E|O | h_hi E|O] x S
        q01 = qkpool.tile([128, S], BF16, tag="q01")
        q23 = qkpool.tile([128, S], BF16, tag="q23")
        k01 = qkpool.tile([128, S], BF16, tag="k01")
        k23 = qkpool.tile([128, S], BF16, tag="k23")
        # V per head padded to 96 rows: [1 | 0*31 | V_h (64)] x4 per chunk.
        # The leading ones-column accumulates the softmax denominator at acc
        # partition 0 (custom-DVE recip needs base partition 0); the zero pad
        # keeps the V rows 32-aligned for the PSUM normalize read.
        vbig = vpool.tile([128, 32 * 512], BF16, tag="vbig")
        nc.vector.memset(vbig[:], 0.0)
        nc.vector.memset(
            vbig[:].rearrange("p (c h f) -> p (c h) f", c=32, h=4)[:, :, 0:1], 1.0
        )

        loop_ctx = tc.For_i(0, loop_n, 1) if loop_n > 1 else None
        if loop_ctx is not None:
            loop_ctx.__enter__()
        with (
            tc.tile_pool(name="projps", bufs=2, space="PSUM") as projps,
            tc.tile_pool(name="sps", bufs=4, space="PSUM") as sps,
            tc.tile_pool(name="accps", bufs=2, space="PSUM") as accps,
        ):

            def proj_units(sb):
                """Return a list of thunks, each a bounded chunk of the
                s-block-sb projection (<=2 projps slots live per thunk)."""
                state = {}

                def load_x():
                    scol = slice(sb * SBLK, (sb + 1) * SBLK)
                    xt = []
                    for i in range(8):
                        t = xpool.tile([128, SBLK], BF16, tag="xt")
                        nc.sync.dma_start(t[:], xT[i * 128 : (i + 1) * 128, scol])
                        xt.append(t)
                    cc_t = trig.tile([128, SBLK], F32, tag="cc")
                    ss_t = trig.tile([128, SBLK], F32, tag="ss")
                    nc.sync.dma_start(cc_t[:], cc4[:, scol])
                    nc.sync.dma_start(ss_t[:], ss4[:, scol])
                    state["xt"] = xt
                    state["cc"] = cc_t
                    state["ss"] = ss_t

                def qk_pair(name, dst):
                    def th():
                        scol = slice(sb * SBLK, (sb + 1) * SBLK)
                        xt, cc_t, ss_t = state["xt"], state["cc"], state["ss"]
                        p1 = projps.tile(
                            [128, SBLK], F32, tag="proj", name=f"p1{sb}_{name}"
                        )
                        p2 = projps.tile(
                            [128, SBLK], F32, tag="proj", name=f"p2{sb}_{name}"
                        )
                        for dstp, w in ((p1, wt[name]), (p2, wt[name + "s"])):
                            for i in range(8):
                                nc.tensor.matmul(
                                    dstp[:],
                                    w[:, i * 128 : (i + 1) * 128],
                                    xt[i][:],
                                    start=(i == 0),
                                    stop=(i == 7),
                                )
                        t1 = tmp.tile([128, SBLK], F32, tag="t1")
                        t2 = tmp.tile([128, SBLK], F32, tag="t2")
                        nc.vector.tensor_mul(t1[:], p1[:], cc_t[:])
                        nc.vector.tensor_mul(t2[:], p2[:], ss_t[:])
                        nc.vector.tensor_add(dst[:, scol], t1[:], t2[:])

                    return th

                def v_unit(ss):
                    def th():
                        xt = state["xt"]
                        vp = projps.tile([128, 256], F32, tag="proj", name=f"vp{sb}_{ss}")
                        for i in range(8):
                            nc.tensor.matmul(
                                vp[:],
                                xt[i][:, ss * 128 : (ss + 1) * 128],
                                w_v[:, i * 256 : (i + 1) * 256],
                                start=(i == 0),
                                stop=(i == 7),
                            )
                        kc = sb * 4 + ss
                        nc.vector.tensor_copy(
                            vbig[:, kc * 512 : (kc + 1) * 512].rearrange(
                                "p (h f) -> p h f", h=4
                            )[:, :, 64:128],
                            vp[:].rearrange("p (h f) -> p h f", h=4),
                        )

                    return th

                units = [load_x]
                for name, dst in (
                    ("q01", q01),
                    ("q23", q23),
                    ("k01", k01),
                    ("k23", k23),
                ):
                    units.append(qk_pair(name, dst))
                for ss in range(4):
                    units.append(v_unit(ss))
                return units

            def emit_attn(qb, a_tiles):
                """Generator: yields after each score/exp/PV group so the
                driver can interleave proj/oproj thunks into the PE stream."""
                nk = 4 * qb + 4
                nfull = 4 * qb + 1  # chunks with d == 0
                for hp in range(2):
                    qt, kt = (q01, k01) if hp == 0 else (q23, k23)
                    heads = (2 * hp, 2 * hp + 1)
                    accs = [
                        accps.tile([128, SBLK], F32, tag="acc", name=f"acc{qb}_{h}")
                        for h in heads
                    ]
                    # chunk schedule: pairs of full chunks -> [128,1024]
                    # activates; leftovers and partial chunks single.
                    c = 0
                    groups = []
                    while c < nk:
                        if c + 1 < nfull:
                            groups.append((c, 2))
                            c += 2
                        else:
                            groups.append((c, 1))
                            c += 1
                    for g0, glen in groups:
                        # always 2-bank tiles so the pool tag has one slot size
                        spb = [
                            sps.tile(
                                [128, 2 * SBLK],
                                F32,
                                tag="s",
                                name=f"s{qb}_{g0}_{h}",
                                bufs=2,
                            )
                            for h in heads
                        ]
                        for gi in range(glen):
                            cch = g0 + gi
                            d = max(0, (cch - 4 * qb)) * KC
                            for hi in range(2):
                                nc.tensor.matmul(
                                    spb[hi][:, gi * SBLK + d : (gi + 1) * SBLK],
                                    kt[hi * 64 : hi * 64 + 64, cch * KC : (cch + 1) * KC],
                                    qt[hi * 64 : hi * 64 + 64, qb * SBLK + d : (qb + 1) * SBLK],
                                    start=True,
                                    stop=True,
                                    tile_position=(hi * 64, 0),
                                )
                        pts = []
                        for hi in range(2):
                            pt = ppool.tile([128, 2 * SBLK], BF16, tag="pt", bufs=8)
                            if glen == 2:
                                nc.scalar.activation(pt[:], spb[hi][:], EXP, scale=0.125)
                            else:
                                d = max(0, (g0 - 4 * qb)) * KC
                                nc.scalar.activation(
                                    pt[:, d:SBLK], spb[hi][:, d:SBLK], EXP, scale=0.125
                                )
                                if g0 >= 4 * qb:
                                    nc.vector.tensor_mul(
                                        pt[:, d : d + KC], pt[:, d : d + KC], mask_t[:]
                                    )
                            pts.append(pt)
                        for gi in range(glen):
                            cch = g0 + gi
                            d = max(0, (cch - 4 * qb)) * KC
                            for hi, h in enumerate(heads):
                                nc.tensor.matmul(
                                    accs[hi][0:128, d:SBLK],
                                    vbig[:, cch * 512 + h * 128 : cch * 512 + (h + 1) * 128],
                                    pts[hi][:, gi * SBLK + d : (gi + 1) * SBLK],
                                    start=(cch == 0),
                                    stop=(cch == nk - 1),
                                )
                        yield
                    for hi, h in enumerate(heads):
                        recip = rpool.tile([1, SBLK], F32, tag="recip")
                        nc.vector.reciprocal_approx_fast(recip[:], accs[hi][0:1, :])
                        rb = rbpool.tile([64, SBLK], F32, tag="rb")
                        nc.gpsimd.partition_broadcast(rb[:], recip[0:1, :])
                        nc.vector.tensor_mul(
                            a_tiles[h // 2][(h % 2) * 64 : (h % 2 + 1) * 64, :],
                            accs[hi][64:128, :],
                            rb[:],
                        )

            def oproj_units(qb, a_tiles):
                def unit(ss, ob):
                    def th():
                        r0 = qb * SBLK + ss * 128
                        op = projps.tile(
                            [128, 512], F32, tag="proj", name=f"op{qb}_{ss}_{ob}"
                        )
                        nc.tensor.matmul(
                            op[:],
                            a_tiles[0][:, ss * 128 : (ss + 1) * 128],
                            w_o0[:, ob * 512 : (ob + 1) * 512],
                            start=True,
                            stop=False,
                        )
                        nc.tensor.matmul(
                            op[:],
                            a_tiles[1][:, ss * 128 : (ss + 1) * 128],
                            w_o1[:, ob * 512 : (ob + 1) * 512],
                            start=False,
                            stop=True,
                        )
                        osb = ppool.tile(
                            [128, 512], BF16, tag="osb", bufs=2, name=f"osb{qb}_{ss}_{ob}"
                        )
                        nc.vector.tensor_copy(osb[:], op[:])
                        nc.sync.dma_start(
                            outp[r0 : r0 + 128, ob * 512 : (ob + 1) * 512], osb[:]
                        )

                    return th

                return [unit(ss, ob) for ss in range(4) for ob in range(2)]

            at_all = [
                [
                    apool.tile([128, SBLK], BF16, tag="a", name=f"a{qb}_{i}", bufs=4)
                    for i in range(2)
                ]
                for qb in range(NSB)
            ]

            def drive(qb, thunks):
                """Emit attention for qb, interleaving `thunks` after groups."""
                gen = emit_attn(qb, at_all[qb])
                ngroups = sum(1 for _ in emit_count(qb))
                pace = len(thunks) / max(ngroups, 1)
                acc = 0.0
                i = 0
                for _ in gen:
                    acc += pace
                    while i < len(thunks) and acc >= 1.0:
                        thunks[i]()
                        i += 1
                        acc -= 1.0
                while i < len(thunks):
                    thunks[i]()
                    i += 1

            def emit_count(qb):
                # mirror of emit_attn's group structure (for pacing)
                nk = 4 * qb + 4
                nfull = 4 * qb + 1
                for hp in range(2):
                    c = 0
                    while c < nk:
                        if c + 1 < nfull:
                            yield
                            c += 2
                        else:
                            yield
                            c += 1

            for th in proj_units(0):
                th()
            drive(0, proj_units(1))
            for sb in range(1, NSB):
                thunks = []
                if sb + 1 < NSB:
                    pu = proj_units(sb + 1)
                    ou = oproj_units(sb - 1, at_all[sb - 1])
                    # alternate so projps never has a pair + op live together
                    # beyond pool capacity pacing: p-pair, op, p-pair, op...
                    while pu or ou:
                        if pu:
                            thunks.append(pu.pop(0))
                        if ou:
                            thunks.append(ou.pop(0))
                else:
                    thunks = oproj_units(sb - 1, at_all[sb - 1])
                drive(sb, thunks)
            for th in oproj_units(NSB - 1, at_all[NSB - 1]):
                th()
        if loop_ctx is not None:
            loop_ctx.__exit__(None, None, None)


def _build(loop_n=1):
    global _PROGRAM
    if loop_n != 1:
        nc = bacc.Bacc(
            "TRN2", target_bir_lowering=False, debug=False, num_devices=N_CORES
        )
        _emit(nc, loop_n)
        nc.compile()
        return nc
    if _PROGRAM is None:
        nc = bacc.Bacc(
            "TRN2", target_bir_lowering=False, debug=False, num_devices=N_CORES
        )
        _emit(nc)
        nc.compile()
        _PROGRAM = nc
    return _PROGRAM


def _rope_caches():
    j = np.arange(0, DK, 2, dtype=np.float32) / np.float32(DK)
    freqs = (1.0 / THETA**j).astype(np.float32)  # [32]
    t = np.arange(S, dtype=np.float32)
    ang = np.outer(t, freqs).astype(np.float32)  # [S, 32]
    return np.cos(ang), np.sin(ang)


def _make_masks():
    kk = np.arange(128)[:, None]
    jj = np.arange(128)[None, :]
    return (kk <= jj).astype(np.float32)


def _to_bf16(a):
    import ml_dtypes

    return np.asarray(a, dtype=np.float32).astype(ml_dtypes.bfloat16)


def _make_in_maps(x, token_positions, Wq, Wk, Wv, Wo):
    x = np.asarray(x, dtype=np.float32)
    token_positions = np.asarray(token_positions)
    Wq, Wk, Wv, Wo = (np.asarray(w, dtype=np.float32) for w in (Wq, Wk, Wv, Wo))

    cos_c, sin_c = _rope_caches()
    masks = _to_bf16(_make_masks())

    in_maps = []
    for c in range(N_CORES):
        b, g = divmod(c, 4)
        heads = [g * HPC + hh for hh in range(HPC)]

        def qk_rows(hpair):
            # interleaved [hE(32) | hO(32)] per head for the two heads
            rows = []
            for h in hpair:
                rows.append(heads[h] * DK + np.arange(0, DK, 2))  # E
                rows.append(heads[h] * DK + np.arange(1, DK, 2))  # O
            return np.concatenate(rows)

        def qk_rows_swap(hpair):
            rows = []
            for h in hpair:
                rows.append(heads[h] * DK + np.arange(1, DK, 2))  # O first
                rows.append(heads[h] * DK + np.arange(0, DK, 2))  # then E
            return np.concatenate(rows)

        rows_v = np.concatenate([h * DK + np.arange(DK) for h in heads])

        pos = np.asarray(token_positions[b], dtype=np.int64)
        cosb = np.ascontiguousarray(cos_c[pos].T)  # [32, S]
        sinb = np.ascontiguousarray(sin_c[pos].T)
        cc = np.tile(cosb, (4, 1))  # every 32-row block: cos
        ss = np.concatenate([-sinb, sinb, -sinb, sinb], axis=0)  # E:-sin O:+sin

        im = {
            "xT": _to_bf16(np.ascontiguousarray(x[b].T)),
            "wvT": _to_bf16(np.ascontiguousarray(Wv[rows_v].T)),
            "woT": _to_bf16(np.ascontiguousarray(Wo[:, rows_v].T)),
            "cc4": np.ascontiguousarray(cc),
            "ss4": np.ascontiguousarray(ss),
            "masks": masks,
        }
        for t, W in (("q", Wq), ("k", Wk)):
            im[f"w{t}01T"] = _to_bf16(np.ascontiguousarray(W[qk_rows((0, 1))].T))
            im[f"w{t}01sT"] = _to_bf16(np.ascontiguousarray(W[qk_rows_swap((0, 1))].T))
            im[f"w{t}23T"] = _to_bf16(np.ascontiguousarray(W[qk_rows((2, 3))].T))
            im[f"w{t}23sT"] = _to_bf16(np.ascontiguousarray(W[qk_rows_swap((2, 3))].T))
        in_maps.append(im)
    return in_maps


def kernel(x, token_positions, Wq, Wk, Wv, Wo):
    nc = _build()
    in_maps = _make_in_maps(x, token_positions, Wq, Wk, Wv, Wo)
    res = run_bass_kernel_spmd(nc, in_maps, list(range(N_CORES)))
    out = np.zeros((B, S, D), dtype=np.float32)
    for c in range(N_CORES):
        out[c // 4] += np.asarray(res.results[c]["out"], dtype=np.float32)
    return out


# revision 12
# speedup vs baseline: 1.2043x; 1.0401x over previous
"""Multi-head self-attention with RoPE on 8 Trainium2 NeuronCores.

Sharding: core c = batch(c // 4) x head-group(c % 4) -> 4 heads per core.
Each core computes attention for its 4 heads and a partial O-projection
(full [S, D] output restricted to its 256 input features); the host sums
the 4 partials per batch (in fp32, from bf16 device partials).

v2 layout (vs v1):
  * Q/K live in two [128, S] tiles per tensor: tile01 = heads {0,1},
    tile23 = {2,3}, rows per head = [E(32) | O(32)] (rope-even dims then
    odd dims). A head's score matmul is then a single 64-contraction
    matmul; the two heads of a tile run concurrently at tile_position
    (0,0)/(64,0) (measured 257ns per head-pair vs 930ns for the v1
    E/O-accumulate pattern).
  * RoPE needs E/O cross-partition arithmetic, which DVE can't do
    lane-misaligned; instead the projection is computed TWICE per tile:
    P1 with the interleaved weight, P2 with its 32-row-block-swapped
    copy (so P2 = swap32(P1), by PE). Then out = P1*cc + P2*ss with
    cc = [cos]*4 rows, ss = [-sin,+sin,-sin,+sin] rows: 3 full-width
    DVE ops, all lane-aligned.
  * exp is issued per-head over PAIRS of full k-chunks as one [128,1024]
    activate spanning 2 PSUM banks (measured 1086ns vs 2x625ns);
    partial (diagonal) chunks keep single [128, 512-d] activates.
  * Scores are computed transposed (S_T[k, q]) so P_T feeds the PV matmul
    as the moving operand; a ones-column appended to V accumulates the
    softmax denominator in the same matmul. Softmax skips the max
    subtraction (scores are bounded ~|5|).
  * All matmul operands are bf16; psum accumulation stays fp32.
  * softmax denominator reciprocal uses the 1-instruction approx
    custom-DVE op; V blocks are padded to 128 rows (ones col first) so
    the denominator lands at acc partition 0.
  * projection(sb), O-projection(sb-1) and attention(sb) are emitted in
    one fused pipelined loop (PSUM: proj 2 banks + scores 2x2 + accs 2).
"""

import os
import sys

sys.path.insert(0, "/opt/trn_rl_repo")

from contextlib import ExitStack

import numpy as np

import concourse.bass as bass
import concourse.tile as tile
from concourse import bacc, mybir
from concourse.bass_utils import run_bass_kernel_spmd

B = 2
S = 4096
D = 1024
NH = 16
DK = 64
HPC = 4  # heads per core
N_CORES = 8
THETA = 10000.0
SBLK = 512  # s-block / q-block width
NSB = S // SBLK
KC = 128  # k chunk
F32 = mybir.dt.float32
BF16 = mybir.dt.bfloat16
EXP = mybir.ActivationFunctionType.Exp

_PROGRAM = None


def _emit(nc, loop_n=1):
    xT = nc.dram_tensor("xT", [D, S], BF16, kind="ExternalInput").ap()
    # interleaved qk projection weights: per head-pair tile, rows =
    # [h_even E32 | h_even O32 | h_odd E32 | h_odd O32]; the *s variants
    # are the 32-row-block-swapped copies.
    wq = {}
    for t in ("q01", "q01s", "q23", "q23s", "k01", "k01s", "k23", "k23s"):
        wq[t] = nc.dram_tensor(f"w{t}T", [D, 128], BF16, kind="ExternalInput").ap()
    wvT = nc.dram_tensor("wvT", [D, 256], BF16, kind="ExternalInput").ap()
    woT = nc.dram_tensor("woT", [256, D], BF16, kind="ExternalInput").ap()
    cc4 = nc.dram_tensor("cc4", [128, S], F32, kind="ExternalInput").ap()
    ss4 = nc.dram_tensor("ss4", [128, S], F32, kind="ExternalInput").ap()
    masks = nc.dram_tensor("masks", [128, 128], BF16, kind="ExternalInput").ap()
    outp = nc.dram_tensor("out", [S, D], BF16, kind="ExternalOutput").ap()

    with tile.TileContext(nc) as tc, ExitStack() as ctx:
        wpool = ctx.enter_context(tc.tile_pool(name="w", bufs=1))
        xpool = ctx.enter_context(tc.tile_pool(name="x", bufs=24))
        qkpool = ctx.enter_context(tc.tile_pool(name="qk", bufs=1))
        vpool = ctx.enter_context(tc.tile_pool(name="v", bufs=1))
        trig = ctx.enter_context(tc.tile_pool(name="trig", bufs=3))
        tmp = ctx.enter_context(tc.tile_pool(name="tmp", bufs=4))
        ppool = ctx.enter_context(tc.tile_pool(name="p", bufs=16))
        apool = ctx.enter_context(tc.tile_pool(name="a", bufs=8))
        rpool = ctx.enter_context(tc.tile_pool(name="r", bufs=4))
        rbpool = ctx.enter_context(tc.tile_pool(name="rb", bufs=4))

        # ---- persistent SBUF tensors ----
        wt = {}
        for t in ("q01", "q01s", "q23", "q23s", "k01", "k01s", "k23", "k23s"):
            wt[t] = wpool.tile([128, 8 * 128], BF16, tag=f"w{t}", name=f"wt_{t}")
            nc.sync.dma_start(
                wt[t][:].rearrange("p (i f) -> p i f", i=8),
                wq[t].rearrange("(i p) f -> p i f", p=128),
            )
        w_v = wpool.tile([128, 8 * 256], BF16, tag="wv")
        nc.sync.dma_start(
            w_v[:].rearrange("p (i f) -> p i f", i=8),
            wvT.rearrange("(i p) f -> p i f", p=128),
        )
        w_o0 = wpool.tile([128, D], BF16, tag="wo0")
        w_o1 = wpool.tile([128, D], BF16, tag="wo1")
        mask_t = wpool.tile([128, 128], BF16, tag="mask")
        nc.sync.dma_start(w_o0[:], woT[0:128, :])
        nc.sync.dma_start(w_o1[:], woT[128:256, :])
        nc.sync.dma_start(mask_t[:], masks)

        # q/k tiles in interleaved layout: [h_lo E|O | h_hi E|O] x S
        q01 = qkpool.tile([128, S], BF16, tag="q01")
        q23 = qkpool.tile([128, S], BF16, tag="q23")
        k01 = qkpool.tile([128, S], BF16, tag="k01")
        k23 = qkpool.tile([128, S], BF16, tag="k23")
        # V per head padded to 96 rows: [1 | 0*31 | V_h (64)] x4 per chunk.
        # The leading ones-column accumulates the softmax denominator at acc
        # partition 0 (custom-DVE recip needs base partition 0); the zero pad
        # keeps the V rows 32-aligned for the PSUM normalize read.
        vbig = vpool.tile([128, 32 * 512], BF16, tag="vbig")
        nc.vector.memset(vbig[:], 0.0)
        nc.vector.memset(
            vbig[:].rearrange("p (c h f) -> p (c h) f", c=32, h=4)[:, :, 0:1], 1.0
        )

        loop_ctx = tc.For_i(0, loop_n, 1) if loop_n > 1 else None
        if loop_ctx is not None:
            loop_ctx.__enter__()
        with (
            tc.tile_pool(name="projps", bufs=2, space="PSUM") as projps,
            tc.tile_pool(name="sps", bufs=4, space="PSUM") as sps,
            tc.tile_pool(name="accps", bufs=2, space="PSUM") as accps,
        ):

            def proj_units(sb):
                """Return a list of thunks, each a bounded chunk of the
                s-block-sb projection (<=2 projps slots live per thunk)."""
                state = {}

                def load_x():
                    scol = slice(sb * SBLK, (sb + 1) * SBLK)
                    xt = []
                    for i in range(8):
                        t = xpool.tile([128, SBLK], BF16, tag="xt")
                        nc.sync.dma_start(t[:], xT[i * 128 : (i + 1) * 128, scol])
                        xt.append(t)
                    cc_t = trig.tile([128, SBLK], F32, tag="cc")
                    ss_t = trig.tile([128, SBLK], F32, tag="ss")
                    nc.sync.dma_start(cc_t[:], cc4[:, scol])
                    nc.sync.dma_start(ss_t[:], ss4[:, scol])
                    state["xt"] = xt
                    state["cc"] = cc_t
                    state["ss"] = ss_t

                def qk_mm(name, which, lo, hi):
                    # one 4-matmul slice of P1 or P2 (alloc on first slice)
                    def th():
                        xt = state["xt"]
                        if which not in state.get(name, {}):
                            state.setdefault(name, {})[which] = projps.tile(
                                [128, SBLK], F32, tag="proj", name=f"{which}{sb}_{name}"
                            )
                        dstp = state[name][which]
                        w = wt[name if which == "p1" else name + "s"]
                        for i in range(lo, hi):
                            nc.tensor.matmul(
                                dstp[:],
                                w[:, i * 128 : (i + 1) * 128],
                                xt[i][:],
                                start=(i == 0),
                                stop=(i == 7),
                            )

                    return th

                def qk_rope(name, dst):
                    def th():
                        scol = slice(sb * SBLK, (sb + 1) * SBLK)
                        cc_t, ss_t = state["cc"], state["ss"]
                        p1 = state[name]["p1"]
                        p2 = state[name]["p2"]
                        t1 = tmp.tile([128, SBLK], F32, tag="t1")
                        t2 = tmp.tile([128, SBLK], F32, tag="t2")
                        nc.vector.tensor_mul(t1[:], p1[:], cc_t[:])
                        nc.vector.tensor_mul(t2[:], p2[:], ss_t[:])
                        nc.vector.tensor_add(dst[:, scol], t1[:], t2[:])

                    return th

                def v_unit(ss, lo, hi):
                    def th():
                        xt = state["xt"]
                        if ss not in state.get("v", {}):
                            state.setdefault("v", {})[ss] = projps.tile(
                                [128, 256], F32, tag="proj", name=f"vp{sb}_{ss}"
                            )
                        vp = state["v"][ss]
                        for i in range(lo, hi):
                            nc.tensor.matmul(
                                vp[:],
                                xt[i][:, ss * 128 : (ss + 1) * 128],
                                w_v[:, i * 256 : (i + 1) * 256],
                                start=(i == 0),
                                stop=(i == 7),
                            )
                        if hi == 8:
                            kc = sb * 4 + ss
                            nc.vector.tensor_copy(
                                vbig[:, kc * 512 : (kc + 1) * 512].rearrange(
                                    "p (h f) -> p h f", h=4
                                )[:, :, 64:128],
                                vp[:].rearrange("p (h f) -> p h f", h=4),
                            )

                    return th

                units = [load_x]
                for name, dst in (
                    ("q01", q01),
                    ("q23", q23),
                    ("k01", k01),
                    ("k23", k23),
                ):
                    units.append(qk_mm(name, "p1", 0, 4))
                    units.append(qk_mm(name, "p1", 4, 8))
                    units.append(qk_mm(name, "p2", 0, 4))
                    units.append(qk_mm(name, "p2", 4, 8))
                    units.append(qk_rope(name, dst))
                for ss in range(4):
                    units.append(v_unit(ss, 0, 4))
                    units.append(v_unit(ss, 4, 8))
                return units

            def attn_groups(qb):
                nk = 4 * qb + 4
                nfull = 4 * qb + 1  # chunks with d == 0
                c = 0
                groups = []
                while c < nk:
                    if c + 1 < nfull:
                        groups.append((c, 2))
                        c += 2
                    else:
                        groups.append((c, 1))
                        c += 1
                return groups

            def emit_attn(qb, a_tiles, pop):
                """Software-pipelined: per group, scores+exp for group g are
                emitted, then PV for group g-1, then ~pace interleave thunks
                (so PE has proj work while ACT runs exp)."""
                nk = 4 * qb + 4
                groups = attn_groups(qb)
                for hp in range(2):
                    qt, kt = (q01, k01) if hp == 0 else (q23, k23)
                    heads = (2 * hp, 2 * hp + 1)
                    accs = [
                        accps.tile([128, SBLK], F32, tag="acc", name=f"acc{qb}_{h}")
                        for h in heads
                    ]

                    def make_pv(g0, glen, pts):
                        def pv():
                            for gi in range(glen):
                                cch = g0 + gi
                                d = max(0, (cch - 4 * qb)) * KC
                                for hi, h in enumerate(heads):
                                    nc.tensor.matmul(
                                        accs[hi][0:128, d:SBLK],
                                        vbig[
                                            :,
                                            cch * 512 + h * 128 : cch * 512 + (h + 1) * 128,
                                        ],
                                        pts[hi][:, gi * SBLK + d : (gi + 1) * SBLK],
                                        start=(cch == 0),
                                        stop=(cch == nk - 1),
                                    )

                        return pv

                    prev_pv = None
                    for g0, glen in groups:
                        spb = [
                            sps.tile(
                                [128, 2 * SBLK],
                                F32,
                                tag="s",
                                name=f"s{qb}_{g0}_{h}",
                                bufs=2,
                            )
                            for h in heads
                        ]
                        for gi in range(glen):
                            cch = g0 + gi
                            d = max(0, (cch - 4 * qb)) * KC
                            for hi in range(2):
                                nc.tensor.matmul(
                                    spb[hi][:, gi * SBLK + d : (gi + 1) * SBLK],
                                    kt[hi * 64 : hi * 64 + 64, cch * KC : (cch + 1) * KC],
                                    qt[
                                        hi * 64 : hi * 64 + 64,
                                        qb * SBLK + d : (qb + 1) * SBLK,
                                    ],
                                    start=True,
                                    stop=True,
                                    tile_position=(hi * 64, 0),
                                )
                        pts = []
                        for hi in range(2):
                            pt = ppool.tile([128, 2 * SBLK], BF16, tag="pt", bufs=8)
                            if glen == 2:
                                nc.scalar.activation(pt[:], spb[hi][:], EXP, scale=0.125)
                            else:
                                d = max(0, (g0 - 4 * qb)) * KC
                                nc.scalar.activation(
                                    pt[:, d:SBLK], spb[hi][:, d:SBLK], EXP, scale=0.125
                                )
                                if g0 >= 4 * qb:
                                    nc.vector.tensor_mul(
                                        pt[:, d : d + KC], pt[:, d : d + KC], mask_t[:]
                                    )
                            pts.append(pt)
                        if prev_pv is not None:
                            prev_pv()
                        pop()
                        prev_pv = make_pv(g0, glen, pts)
                    prev_pv()
                    for hi, h in enumerate(heads):
                        recip = rpool.tile([1, SBLK], F32, tag="recip")
                        nc.vector.reciprocal_approx_fast(recip[:], accs[hi][0:1, :])
                        rb = rbpool.tile([64, SBLK], F32, tag="rb")
                        nc.gpsimd.partition_broadcast(rb[:], recip[0:1, :])
                        nc.vector.tensor_mul(
                            a_tiles[h // 2][(h % 2) * 64 : (h % 2 + 1) * 64, :],
                            accs[hi][64:128, :],
                            rb[:],
                        )

            def oproj_units(qb, a_tiles):
                def unit(ss, ob):
                    def th():
                        r0 = qb * SBLK + ss * 128
                        op = projps.tile(
                            [128, 512], F32, tag="proj", name=f"op{qb}_{ss}_{ob}"
                        )
                        nc.tensor.matmul(
                            op[:],
                            a_tiles[0][:, ss * 128 : (ss + 1) * 128],
                            w_o0[:, ob * 512 : (ob + 1) * 512],
                            start=True,
                            stop=False,
                        )
                        nc.tensor.matmul(
                            op[:],
                            a_tiles[1][:, ss * 128 : (ss + 1) * 128],
                            w_o1[:, ob * 512 : (ob + 1) * 512],
                            start=False,
                            stop=True,
                        )
                        osb = ppool.tile(
                            [128, 512], BF16, tag="osb", bufs=2, name=f"osb{qb}_{ss}_{ob}"
                        )
                        nc.vector.tensor_copy(osb[:], op[:])
                        nc.sync.dma_start(
                            outp[r0 : r0 + 128, ob * 512 : (ob + 1) * 512], osb[:]
                        )

                    return th

                return [unit(ss, ob) for ss in range(4) for ob in range(2)]

            at_all = [
                [
                    apool.tile([128, SBLK], BF16, tag="a", name=f"a{qb}_{i}", bufs=4)
                    for i in range(2)
                ]
                for qb in range(NSB)
            ]

            def block_thunks(pu, ou):
                """Order proj-unit thunks and oproj thunks so that projps
                never holds a qk pair (2 slots) plus an op tile at once:
                op thunks go between complete qk-pair windows / v units.
                pu layout: [load_x, (qk x5)*4, (v x2)*4]; ou: 8 op thunks."""
                if pu is None:
                    return list(ou)
                out = [pu[0]]
                ou = list(ou)
                for i in range(4):  # qk pairs: 5 thunks each
                    out.extend(pu[1 + 5 * i : 1 + 5 * (i + 1)])
                    if ou:
                        out.append(ou.pop(0))
                for i in range(4):  # v units: 2 thunks each
                    out.extend(pu[21 + 2 * i : 21 + 2 * (i + 1)])
                    if ou:
                        out.append(ou.pop(0))
                out.extend(ou)
                return out

            def drive(qb, thunks):
                ngroups = 2 * len(attn_groups(qb))
                pace = len(thunks) / max(ngroups, 1)
                state = {"acc": 0.0, "i": 0}

                def pop():
                    state["acc"] += pace
                    while state["i"] < len(thunks) and state["acc"] >= 1.0:
                        thunks[state["i"]]()
                        state["i"] += 1
                        state["acc"] -= 1.0

                emit_attn(qb, at_all[qb], pop)
                while state["i"] < len(thunks):
                    thunks[state["i"]]()
                    state["i"] += 1

            for th in proj_units(0):
                th()
            drive(0, proj_units(1))
            for sb in range(1, NSB):
                if sb + 1 < NSB:
                    thunks = block_thunks(
                        proj_units(sb + 1), oproj_units(sb - 1, at_all[sb - 1])
                    )
                else:
                    thunks = block_thunks(None, oproj_units(sb - 1, at_all[sb - 1]))
                drive(sb, thunks)
            for th in oproj_units(NSB - 1, at_all[NSB - 1]):
                th()
        if loop_ctx is not None:
            loop_ctx.__exit__(None, None, None)


def _build(loop_n=1):
    global _PROGRAM
    if loop_n != 1:
        nc = bacc.Bacc(
            "TRN2", target_bir_lowering=False, debug=False, num_devices=N_CORES
        )
        _emit(nc, loop_n)
        nc.compile()
        return nc
    if _PROGRAM is None:
        nc = bacc.Bacc(
            "TRN2", target_bir_lowering=False, debug=False, num_devices=N_CORES
        )
        _emit(nc)
        nc.compile()
        _PROGRAM = nc
    return _PROGRAM


def _rope_caches():
    j = np.arange(0, DK, 2, dtype=np.float32) / np.float32(DK)
    freqs = (1.0 / THETA**j).astype(np.float32)  # [32]
    t = np.arange(S, dtype=np.float32)
    ang = np.outer(t, freqs).astype(np.float32)  # [S, 32]
    return np.cos(ang), np.sin(ang)


def _make_masks():
    kk = np.arange(128)[:, None]
    jj = np.arange(128)[None, :]
    return (kk <= jj).astype(np.float32)


def _to_bf16(a):
    import ml_dtypes

    return np.asarray(a, dtype=np.float32).astype(ml_dtypes.bfloat16)


def _make_in_maps(x, token_positions, Wq, Wk, Wv, Wo):
    x = np.asarray(x, dtype=np.float32)
    token_positions = np.asarray(token_positions)
    Wq, Wk, Wv, Wo = (np.asarray(w, dtype=np.float32) for w in (Wq, Wk, Wv, Wo))

    cos_c, sin_c = _rope_caches()
    masks = _to_bf16(_make_masks())

    in_maps = []
    for c in range(N_CORES):
        b, g = divmod(c, 4)
        heads = [g * HPC + hh for hh in range(HPC)]

        def qk_rows(hpair):
            # interleaved [hE(32) | hO(32)] per head for the two heads
            rows = []
            for h in hpair:
                rows.append(heads[h] * DK + np.arange(0, DK, 2))  # E
                rows.append(heads[h] * DK + np.arange(1, DK, 2))  # O
            return np.concatenate(rows)

        def qk_rows_swap(hpair):
            rows = []
            for h in hpair:
                rows.append(heads[h] * DK + np.arange(1, DK, 2))  # O first
                rows.append(heads[h] * DK + np.arange(0, DK, 2))  # then E
            return np.concatenate(rows)

        rows_v = np.concatenate([h * DK + np.arange(DK) for h in heads])

        pos = np.asarray(token_positions[b], dtype=np.int64)
        cosb = np.ascontiguousarray(cos_c[pos].T)  # [32, S]
        sinb = np.ascontiguousarray(sin_c[pos].T)
        cc = np.tile(cosb, (4, 1))  # every 32-row block: cos
        ss = np.concatenate([-sinb, sinb, -sinb, sinb], axis=0)  # E:-sin O:+sin

        im = {
            "xT": _to_bf16(np.ascontiguousarray(x[b].T)),
            "wvT": _to_bf16(np.ascontiguousarray(Wv[rows_v].T)),
            "woT": _to_bf16(np.ascontiguousarray(Wo[:, rows_v].T)),
            "cc4": np.ascontiguousarray(cc),
            "ss4": np.ascontiguousarray(ss),
            "masks": masks,
        }
        for t, W in (("q", Wq), ("k", Wk)):
            im[f"w{t}01T"] = _to_bf16(np.ascontiguousarray(W[qk_rows((0, 1))].T))
            im[f"w{t}01sT"] = _to_bf16(np.ascontiguousarray(W[qk_rows_swap((0, 1))].T))
            im[f"w{t}23T"] = _to_bf16(np.ascontiguousarray(W[qk_rows((2, 3))].T))
            im[f"w{t}23sT"] = _to_bf16(np.ascontiguousarray(W[qk_rows_swap((2, 3))].T))
        in_maps.append(im)
    return in_maps


def kernel(x, token_positions, Wq, Wk, Wv, Wo):
    nc = _build()
    in_maps = _make_in_maps(x, token_positions, Wq, Wk, Wv, Wo)
    res = run_bass_kernel_spmd(nc, in_maps, list(range(N_CORES)))
    out = np.zeros((B, S, D), dtype=np.float32)
    for c in range(N_CORES):
        out[c // 4] += np.asarray(res.results[c]["out"], dtype=np.float32)
    return out
